# revision 21
# baseline (speedup 1.0000x reference)
"""Multi-head causal attention (scores = K @ Q^T variant) on 8 TRN2 NeuronCores.

Head-parallel sharding: core c computes heads (2c, 2c+1); host concatenates.

v2 optimizations over the 214us baseline:
  - AV matmuls for off-diagonal j-blocks run in fp8e4 DoubleRow mode, pairing
    two j-blocks per matmul (V natural layout [128, jb, 64] is already the
    required [K, 2, M] shape).  The softmax denominator comes from a separate
    M=1 ones-matmul into partition 64 of the same PSUM bank (DoubleRow lhsT
    free dim is capped at 128, so the fused [V | 1] M=65 trick cannot pair).
  - All exp outputs are scaled by 2^-5 (bias -5*ln2 folded into the Exp
    activation); the scale cancels in num/den and keeps exp values inside
    fp8e4 range (max causal logit is 8.3; representable max ln(240*32)=8.95).
  - A tunable fraction of off-diagonal j-block pairs evaluate exp on the DVE
    via the Schraudolph bit trick (int16(s*a+b) bitcast to bf16, one
    tensor_scalar), relieving the Act engine which is otherwise a hard
    1 elem/lane/cycle bottleneck (144 exps = 160us).  Those blocks use the
    bf16 M=65 AV path.
  - Scores optionally run in fp8 DoubleRow ([32, 2, .] packing of the 64-dim
    contraction, head0 at partitions 0-31 / head1 at 64-95 to satisfy the
    tile_position == lhsT base partition rule).  Q/K fp8 copies are
    partition-reshaped via SBUF->SBUF DMA.  i-block 0 stays bf16.
  - Diagonal blocks: scores/exp/AV trimmed to the causal i-range
    (i >= 128*q), mask applied post-exp by a gpsimd affine_select, AV stays
    bf16 (fp8 V would corrupt early rows where out ~= v exactly).
"""

import numpy as np

T, D, H, HS = 4096, 1024, 16, 64
NCORES = 8
HPC = H // NCORES  # heads per core = 2
DC = D // 128      # 8 contraction chunks
TC = T // 512      # 8 t-chunks for projections
IB = T // 512      # 8 i-blocks (512 output rows each)
JBN = T // 128     # 32 j-blocks (128 keys each)

# fp8 DoubleRow was measured SLOWER than bf16 here: with K <= 128 partitions the
# PE streams one rhs element per cycle regardless of dtype, so fp8 packing of
# the contraction buys nothing (493ns vs 402ns per 512-col matmul).  bf16
# everywhere; the win comes from the Act/DVE exp split + causal trimming.
DVE_PAIR_MOD = (1, 4)      # off-diag pairs of late i-blocks with jp % 8 here -> DVE

LOG2E = 1.4426950408889634
LN2 = 0.6931471805599453
C_SHIFT = 5.0              # es scaled by 2^-5 everywhere (cancels in num/den)
SC = 0.125                 # 1/sqrt(HS)
ACT_BIAS = -C_SHIFT * LN2
A_TRICK = 128.0 * LOG2E * SC          # psum fp32 -> int16 exponent scale
B_TRICK = (127.0 - 0.0579 - C_SHIFT) * 128.0

_cached_nc = None


def _emit(tc, nc, xh, w6qk, w6v, out):
    import concourse.bass as bass  # noqa: F401
    import concourse.mybir as mybir

    f32 = mybir.dt.float32
    bf16 = mybir.dt.bfloat16
    f8 = mybir.dt.float8e4
    i16 = mybir.dt.int16
    Exp = mybir.ActivationFunctionType.Exp
    ne = mybir.AluOpType.not_equal
    ge = mybir.AluOpType.is_ge
    mult = mybir.AluOpType.mult
    add = mybir.AluOpType.add
    DR = mybir.MatmulPerfMode.DoubleRow

    with (
        tc.tile_pool(name="const", bufs=1) as constp,
        tc.tile_pool(name="wpool", bufs=1) as wpool,
        tc.tile_pool(name="bigp", bufs=1) as bigp,
        tc.tile_pool(name="xpool", bufs=3) as xpool,
        tc.tile_pool(name="vtp", bufs=2) as vtp,
        tc.tile_pool(name="esp", bufs=8) as esp,
        tc.tile_pool(name="finp", bufs=4) as finp,
        tc.tile_pool(name="sp", bufs=2, space="PSUM") as sp,
        tc.tile_pool(name="op", bufs=2, space="PSUM") as op,
        tc.tile_pool(name="pp", bufs=2, space="PSUM") as pp,
    ):
        # ---- input DMAs for weights + first x chunk go first ------------
        # Host pre-lays-out weights and x so every DMA is contiguous per
        # partition (the strided w6 slices measured ~26GB/s).  Small
        # critical slices (dc 0-1) first, on separate engine queues.
        wqk = wpool.tile([128, DC, 256], bf16, tag="wqk", name="wqk")
        wv = wpool.tile([128, DC, 128], bf16, tag="wv", name="wv")
        xts = []
        xt0 = xpool.tile([128, DC, 512], bf16, tag="xt", name="xt0")
        # Cross-queue DMA transfers execute roughly in ISSUE order, so the
        # critical first slices must be issued before any bulk: sync issues
        # earliest (x dc0-1), scalar next (Q/K weights), gpsimd the rest.
        nc.sync.dma_start(out=xt0[:, 0:2, :], in_=xh[:, 0, 0:2, :])
        nc.scalar.dma_start(out=wqk[:, 0:2, :], in_=w6qk[:, 0:2, :])
        nc.sync.dma_start(out=xt0[:, 2:4, :], in_=xh[:, 0, 2:4, :])
        nc.scalar.dma_start(out=wqk[:, 2:DC, :], in_=w6qk[:, 2:DC, :])
        nc.sync.dma_start(out=xt0[:, 4:DC, :], in_=xh[:, 0, 4:DC, :])
        nc.sync.dma_start(out=wv, in_=w6v)
        xts.append(xt0)

        # ---- constants (gpsimd; overlaps the DMAs) ----------------------
        # warm tile first: the PE warmup matmuls wait only on this memset
        warm = constp.tile([128, 512], bf16)
        nc.gpsimd.memset(warm, 0.0)
        id64 = constp.tile([128, 64], bf16)
        nc.gpsimd.memset(id64, 0.0)
        nc.gpsimd.affine_select(
            out=id64, in_=id64, compare_op=ne, fill=1.0,
            base=0, channel_multiplier=1, pattern=[[-1, 64]],
        )
        nc.gpsimd.affine_select(
            out=id64, in_=id64, compare_op=ne, fill=1.0,
            base=-64, channel_multiplier=1, pattern=[[-1, 64]],
        )
        biasap = constp.tile([128, 1], f32)
        nc.gpsimd.memset(biasap, ACT_BIAS)

        # ---- PE warmup during the input-DMA wait -----------------------
        # The PE drops to 1.2GHz after ~any idle and takes ~3us of
        # continuous work to reach 2.4GHz; dummy matmuls on a zeroed
        # scratch tile ramp it while the first x/w slices stream in.
        wps = pp.tile([64, 256], f32, tag="p", name="warm_ps")
        for _ in range(24):
            nc.tensor.matmul(wps, lhsT=warm[:, 0:64], rhs=warm[:, 0:256],
                             start=True, stop=True)

        # ---- persistent activations ------------------------------------
        QT = bigp.tile([128, T], bf16)
        KT = bigp.tile([128, T], bf16)
        Vext0 = bigp.tile([128, JBN, HS + 1], bf16)
        Vext1 = bigp.tile([128, JBN, HS + 1], bf16)
        onesb = constp.tile([128, JBN], bf16)
        nc.gpsimd.memset(onesb, 1.0)
        nc.vector.tensor_copy(Vext0[:, :, HS], onesb)
        nc.vector.tensor_copy(Vext1[:, :, HS], onesb)

        def prefetch_chunk(tcj):
            if tcj < TC and len(xts) <= tcj:
                xtn = xpool.tile([128, DC, 512], bf16, tag="xt", name=f"xt{tcj}")
                nc.sync.dma_start(out=xtn[:, 0:4, :], in_=xh[:, tcj, 0:4, :])
                nc.sync.dma_start(out=xtn[:, 4:8, :], in_=xh[:, tcj, 4:8, :])
                xts.append(xtn)

        def emit_proj_chunk(tcj):
            ts = slice(tcj * 512, (tcj + 1) * 512)
            # prefetch two chunks ahead: chunk k's ~5us of projection matmuls
            # barely covers one 1MB x-chunk DMA, so one-ahead slips ~1us/chunk.
            # (Chunk 0 prefetches only one ahead: its 1MB must not contend
            # with the critical startup slices.)
            prefetch_chunk(tcj + 1)
            if tcj >= 1:
                prefetch_chunk(tcj + 2)
            xt = xts[tcj]
            # Q and K interleaved over dc so each x dc-slice is consumed as
            # its DMA lands (matters for the DMA-gated first chunks).
            ps0 = pp.tile([128, 512], f32, tag="p", name=f"ps0_{tcj}")
            ps1 = pp.tile([128, 512], f32, tag="p", name=f"ps1_{tcj}")
            for dc in range(DC):
                for fc, ps in ((0, ps0), (1, ps1)):
                    nc.tensor.matmul(
                        ps,
                        lhsT=wqk[:, dc, fc * 128:(fc + 1) * 128],
                        rhs=xt[:, dc, :],
                        start=(dc == 0), stop=(dc == DC - 1),
                    )
            # PSUM->SBUF casts ride the Act engine: DVE is the busier
            # exp partner (1218ns/block vs Act's 1007ns).
            nc.scalar.copy(QT[:, ts], ps0)
            nc.scalar.copy(KT[:, ts], ps1)
            psv = pp.tile([128, 512], f32, tag="p", name=f"psv_{tcj}")
            for dc in range(DC):
                nc.tensor.matmul(
                    psv,
                    lhsT=wv[:, dc, :],
                    rhs=xt[:, dc, :],
                    start=(dc == 0), stop=(dc == DC - 1),
                )
            vts = vtp.tile([128, 512], bf16, tag="vts", name=f"vts_{tcj}")
            nc.scalar.copy(vts, psv)
            for h in range(HPC):
                vdst = Vext0 if h == 0 else Vext1
                ptv4 = pp.tile([128, 4, 64], bf16, tag="p", name=f"ptv{h}_{tcj}")
                for q in range(4):
                    nc.tensor.transpose(
                        ptv4[:, q, :],
                        in_=vts[h * 64:(h + 1) * 64, q * 128:(q + 1) * 128],
                        identity=id64[h * 64:(h + 1) * 64, :],
                        tile_position=(h * 64, 0),
                    )
                # one strided copy for all four j-blocks of this chunk
                nc.vector.tensor_copy(
                    vdst[:, tcj * 4:tcj * 4 + 4, 0:HS], ptv4)

        def emit_scores(ps, ib, jb, i0, w, name):
            """Both heads' S^T[j, i0:i0+w] for j-block jb into ps[:, h, i0:]."""
            gi0 = ib * 512 + i0
            for h in range(HPC):
                nc.tensor.matmul(
                    ps[:, h, i0:i0 + w],
                    lhsT=QT[h * 64:(h + 1) * 64, jb * 128:(jb + 1) * 128],
                    rhs=KT[h * 64:(h + 1) * 64, gi0:gi0 + w],
                    start=True, stop=True,
                    tile_position=(h * 64, 0),
                )

        def emit_attn_block(ib):
            isl = slice(ib * 512, (ib + 1) * 512)
            po = [
                op.tile([128, 512], f32, tag="o", name=f"po{h}_{ib}")
                for h in range(HPC)
            ]
            n_blocks = 4 * (ib + 1)
            state = {"started": False, "pending": [], "idx": 0}

            def flush_pending():
                for av in state["pending"]:
                    av()
                state["pending"] = []

            def push_pending(av):
                # Defer each AV two blocks behind its scores: the exp engine
                # gets ~2.7us of slack instead of ~100ns, so the PE never
                # waits on an exp in steady state.
                if len(state["pending"]) >= 3:
                    state["pending"].pop(0)()
                state["pending"].append(av)

            def do_block(jb, use_dve):
                ps = sp.tile([128, 2, 512], f32, tag="s", name=f"s_{ib}_{jb}")
                emit_scores(ps, ib, jb, 0, 512, f"s{ib}_{jb}")
                es = esp.tile([128, 2, 512], bf16, tag="es", name=f"es_{ib}_{jb}")
                if use_dve:
                    nc.vector.tensor_scalar(es.bitcast(i16), ps, A_TRICK,
                                            B_TRICK, mult, add)
                else:
                    nc.scalar.activation(es, ps, Exp, scale=SC, bias=biasap)
                first = not state["started"]
                state["started"] = True
                last = state["idx"] == n_blocks - 1
                state["idx"] += 1

                def av():
                    for h in range(HPC):
                        vdst = Vext0 if h == 0 else Vext1
                        nc.tensor.matmul(
                            po[h][0:65, :],
                            lhsT=vdst[:, jb, :],
                            rhs=es[:, h, :],
                            start=first, stop=last,
                        )
                push_pending(av)

            def do_diag(q, use_dve=False):
                jb = 4 * ib + q
                i0 = 128 * q
                w = 512 - i0
                ps = sp.tile([128, 2, 512], f32, tag="s", name=f"s_{ib}_{jb}")
                emit_scores(ps, ib, jb, i0, w, f"s{ib}_{jb}")
                es = esp.tile([128, 2, 512], bf16, tag="es", name=f"es_{ib}_{jb}")
                if use_dve:
                    nc.vector.tensor_scalar(es.bitcast(i16)[:, :, i0:],
                                            ps[:, :, i0:], A_TRICK, B_TRICK,
                                            mult, add)
                else:
                    nc.scalar.activation(es[:, :, i0:], ps[:, :, i0:], Exp,
                                         scale=SC, bias=biasap)
                # causal wedge: keep iff (i0 + ii) >= j + 128q  <=>  ii >= j
                nc.gpsimd.affine_select(
                    out=es[:, :, i0:], in_=es[:, :, i0:], compare_op=ge,
                    fill=0.0, base=0, channel_multiplier=-1,
                    pattern=[[0, 2], [1, w]],
                )
                first = not state["started"]
                state["started"] = True
                last = state["idx"] == n_blocks - 1
                state["idx"] += 1

                def av():
                    for h in range(HPC):
                        vdst = Vext0 if h == 0 else Vext1
                        nc.tensor.matmul(
                            po[h][0:65, i0:],
                            lhsT=vdst[:, jb, :],
                            rhs=es[:, h, i0:],
                            start=first, stop=last,
                        )
                push_pending(av)

            def offdiag(jp0=0, jp1=None):
                # Strict per-block Act/DVE alternation: both exp engines run
                # concurrently every pair, so the combined exp rate (~1.9
                # elem/ns) always exceeds the PE's ~1.1 elem/ns demand and the
                # PE never throttles to a single engine's pace.
                if jp1 is None:
                    jp1 = 2 * ib
                for jp in range(jp0, jp1):
                    do_block(2 * jp, use_dve=False)
                    do_block(2 * jp + 1, use_dve=True)

            def diag():
                for q in range(4):
                    do_diag(q, use_dve=(q % 2 == 1))

            def finish():
                flush_pending()
                attn_finish_tail(ib, po, state, flush_pending)

            return offdiag, diag, finish

        def attn_finish_tail(ib, po, state, flush_pending):
            for h in range(HPC):
                ot = finp.tile([65, 512], f32, tag="ot", name=f"ot{h}_{ib}")
                # h0 on Act, h1 on DVE; halves pipelined with the DMAs, each
                # head's DMAs on its own queue (sync / gpsimd) so issues
                # overlap.
                cp = nc.scalar.copy if h == 0 else nc.vector.tensor_copy
                dq = nc.sync if h == 0 else nc.scalar
                for q in range(2):
                    fsl = slice(q * 256, (q + 1) * 256)
                    osl = slice(ib * 512 + q * 256, ib * 512 + (q + 1) * 256)
                    cp(ot[:, fsl], po[h][0:65, fsl])
                    dq.dma_start(
                        out=out[h * 65:(h + 1) * 65, osl], in_=ot[:, fsl])

        # Staircase: attention block k needs projection chunks <= k (diag
        # needs exactly chunk k).  Emit proj chunk k+1 between the off-diag
        # and diag phases of attention block k: the PE chews projection
        # matmuls while the Act engine catches up on the off-diag exps,
        # instead of stalling behind the last pending AV.
        emit_proj_chunk(0)
        for k in range(TC):
            offdiag_k, diag_k, finish_k = emit_attn_block(k)
            if k + 1 < TC:
                # Proj chunk k+1 BEFORE block k's attention: its QT/KT/vts
                # copies land at the front of the Act queue, so block k+1's
                # scores never wait on them behind block k's exps.
                emit_proj_chunk(k + 1)
                offdiag_k()
                diag_k()
            else:
                # Last block: two off-diag pairs first (PE work for the
                # serial diag score->exp->mask->AV chain to hide behind),
                # then the diagonal, then the remaining pairs -- so neither
                # the diag chain nor the diag masks stick out as tail.
                offdiag_k(0, 2)
                diag_k()
                offdiag_k(2, 2 * k)
            finish_k()


_NO_HOIST_TYPES = frozenset({"InstNoOp"})


def _pair_ldweights(nc):
    """Reorder LDW0,MM0,LDW1,MM1 -> LDW0,LDW1,MM0,MM1 for row-group pairs."""
    for f in nc.m.functions:
        for blk in f.blocks:
            insts = blk.instructions
            changed = False
            i = 0
            while i + 3 < len(insts):
                a, b, c, d = insts[i:i + 4]
                if (
                    type(a).__name__ == "InstLdweights"
                    and type(b).__name__ == "InstMatmult"
                    and type(c).__name__ == "InstLdweights"
                    and type(d).__name__ == "InstMatmult"
                    and b.tile_position is not None
                    and c.tile_position is not None
                    and b.tile_position[0] == 0
                    and c.tile_position[0] == 64
                    and b.tile_size is not None
                    and b.tile_size[0] <= 64
                ):
                    insts[i + 1], insts[i + 2] = c, b
                    changed = True
                    i += 4
                else:
                    i += 1
            if changed:
                blk.instructions = insts


def _legalize_waits(nc):
    """Hoist multi-waits off engine instructions onto preceding NoOps."""
    import bass_rust

    for f in nc.m.functions:
        for blk in f.blocks:
            out = []
            changed = False
            for inst in blk.instructions:
                si = getattr(inst, "sync_info", None)
                if (
                    type(inst).__name__ not in _NO_HOIST_TYPES
                    and si is not None
                    and len(si.on_wait) >= 2
                ):
                    waits = list(si.on_wait)
                    for k, w in enumerate(waits[:-1]):
                        nop = bass_rust.InstNoOp(name=f"{inst.name}_hoistw{k}")
                        nop.engine = inst.engine
                        nop.sync_info = bass_rust.SyncInfo(
                            on_wait=[w], on_update=[]
                        )
                        out.append(nop)
                    si.on_wait = [waits[-1]]
                    changed = True
                out.append(inst)
            if changed:
                blk.instructions = out


def _build_program():
    import concourse.bass as bass
    import concourse.mybir as mybir
    import concourse.tile as tile

    nc = bass.Bass("TRN2", target_bir_lowering=False, debug=False, num_devices=NCORES)
    xh = nc.dram_tensor("xh", [128, TC, DC, 512], mybir.dt.bfloat16, kind="ExternalInput").ap()
    w6qk = nc.dram_tensor("w6qk", [128, DC, 256], mybir.dt.bfloat16, kind="ExternalInput").ap()
    w6v = nc.dram_tensor("w6v", [128, DC, 128], mybir.dt.bfloat16, kind="ExternalInput").ap()
    out = nc.dram_tensor("outR", [HPC * (HS + 1), T], mybir.dt.float32, kind="ExternalOutput").ap()

    with tile.TileContext(nc) as tc:
        _emit(tc, nc, xh, w6qk, w6v, out)
    _pair_ldweights(nc)
    _legalize_waits(nc)
    return nc


def _in_maps(x, Wk, Wq, Wv):
    import ml_dtypes

    bf = ml_dtypes.bfloat16
    # x host layout [128, TC, DC, 512]: xh[p, tc, dc, t] = x[tc*512+t, dc*128+p]
    xf = np.asarray(x, dtype=np.float32).astype(bf)
    xh = np.ascontiguousarray(
        xf.reshape(TC, 512, DC, 128).transpose(3, 0, 2, 1))
    maps = []
    for c in range(NCORES):
        h0, h1 = HPC * c, HPC * c + 1
        WQK = np.concatenate(
            [Wq[h0], Wq[h1], Wk[h0], Wk[h1]], axis=1).astype(bf)
        WV = np.concatenate([Wv[h0], Wv[h1]], axis=1).astype(bf)
        # [D, F] -> [128, DC, F]: w[p, dc, f] = W[dc*128+p, f]
        wqk = np.ascontiguousarray(WQK.reshape(DC, 128, 256).transpose(1, 0, 2))
        wv = np.ascontiguousarray(WV.reshape(DC, 128, 128).transpose(1, 0, 2))
        maps.append({"xh": xh, "w6qk": wqk, "w6v": wv})
    return maps


def get_program():
    global _cached_nc
    if _cached_nc is None:
        _cached_nc = _build_program()
    return _cached_nc


def kernel(x, Wk, Wq, Wv):
    import os

    from concourse.bass_utils import run_bass_kernel_spmd

    os.environ.setdefault("NEURON_FORCE_RECOMPILE", "1")
    os.environ.setdefault("NEURON_RT_RESET_CORES", "1")

    nc = get_program()
    res = run_bass_kernel_spmd(nc, _in_maps(x, Wk, Wq, Wv), core_ids=list(range(NCORES)))
    cols = []
    for c in range(NCORES):
        raw = res.results[c]["outR"]  # [2*65, T]: per head 64 rows O^T + denom
        for h in range(HPC):
            o = raw[h * 65:h * 65 + HS]
            den = raw[h * 65 + HS:h * 65 + HS + 1]
            cols.append((o / den).T)
    return np.ascontiguousarray(np.concatenate(cols, axis=1), dtype=np.float32)



# revision 22
# speedup vs baseline: 1.0261x; 1.0261x over previous
"""Multi-head causal attention (scores = K @ Q^T variant) on 8 TRN2 NeuronCores.

Head-parallel sharding: core c computes heads (2c, 2c+1); host concatenates.

v2 optimizations over the 214us baseline:
  - AV matmuls for off-diagonal j-blocks run in fp8e4 DoubleRow mode, pairing
    two j-blocks per matmul (V natural layout [128, jb, 64] is already the
    required [K, 2, M] shape).  The softmax denominator comes from a separate
    M=1 ones-matmul into partition 64 of the same PSUM bank (DoubleRow lhsT
    free dim is capped at 128, so the fused [V | 1] M=65 trick cannot pair).
  - All exp outputs are scaled by 2^-5 (bias -5*ln2 folded into the Exp
    activation); the scale cancels in num/den and keeps exp values inside
    fp8e4 range (max causal logit is 8.3; representable max ln(240*32)=8.95).
  - A tunable fraction of off-diagonal j-block pairs evaluate exp on the DVE
    via the Schraudolph bit trick (int16(s*a+b) bitcast to bf16, one
    tensor_scalar), relieving the Act engine which is otherwise a hard
    1 elem/lane/cycle bottleneck (144 exps = 160us).  Those blocks use the
    bf16 M=65 AV path.
  - Scores optionally run in fp8 DoubleRow ([32, 2, .] packing of the 64-dim
    contraction, head0 at partitions 0-31 / head1 at 64-95 to satisfy the
    tile_position == lhsT base partition rule).  Q/K fp8 copies are
    partition-reshaped via SBUF->SBUF DMA.  i-block 0 stays bf16.
  - Diagonal blocks: scores/exp/AV trimmed to the causal i-range
    (i >= 128*q), mask applied post-exp by a gpsimd affine_select, AV stays
    bf16 (fp8 V would corrupt early rows where out ~= v exactly).
"""

import numpy as np

T, D, H, HS = 4096, 1024, 16, 64
NCORES = 8
HPC = H // NCORES  # heads per core = 2
DC = D // 128      # 8 contraction chunks
TC = T // 512      # 8 t-chunks for projections
IB = T // 512      # 8 i-blocks (512 output rows each)
JBN = T // 128     # 32 j-blocks (128 keys each)

# fp8 DoubleRow was measured SLOWER than bf16 here: with K <= 128 partitions the
# PE streams one rhs element per cycle regardless of dtype, so fp8 packing of
# the contraction buys nothing (493ns vs 402ns per 512-col matmul).  bf16
# everywhere; the win comes from the Act/DVE exp split + causal trimming.
DVE_PAIR_MOD = (1, 4)      # off-diag pairs of late i-blocks with jp % 8 here -> DVE

LOG2E = 1.4426950408889634
LN2 = 0.6931471805599453
C_SHIFT = 5.0              # es scaled by 2^-5 everywhere (cancels in num/den)
SC = 0.125                 # 1/sqrt(HS)
ACT_BIAS = -C_SHIFT * LN2
A_TRICK = 128.0 * LOG2E * SC          # psum fp32 -> int16 exponent scale
B_TRICK = (127.0 - 0.0579 - C_SHIFT) * 128.0

_cached_nc = None


def _emit(tc, nc, xh, w6qk, w6v, out):
    import concourse.bass as bass  # noqa: F401
    import concourse.mybir as mybir

    f32 = mybir.dt.float32
    bf16 = mybir.dt.bfloat16
    f8 = mybir.dt.float8e4
    i16 = mybir.dt.int16
    Exp = mybir.ActivationFunctionType.Exp
    ne = mybir.AluOpType.not_equal
    ge = mybir.AluOpType.is_ge
    mult = mybir.AluOpType.mult
    add = mybir.AluOpType.add
    DR = mybir.MatmulPerfMode.DoubleRow

    with (
        tc.tile_pool(name="const", bufs=1) as constp,
        tc.tile_pool(name="wpool", bufs=1) as wpool,
        tc.tile_pool(name="bigp", bufs=1) as bigp,
        tc.tile_pool(name="xpool", bufs=3) as xpool,
        tc.tile_pool(name="vtp", bufs=2) as vtp,
        tc.tile_pool(name="esp", bufs=8) as esp,
        tc.tile_pool(name="finp", bufs=4) as finp,
        tc.tile_pool(name="sp", bufs=2, space="PSUM") as sp,
        tc.tile_pool(name="op", bufs=2, space="PSUM") as op,
        tc.tile_pool(name="pp", bufs=2, space="PSUM") as pp,
    ):
        # ---- input DMAs for weights + first x chunk go first ------------
        # Host pre-lays-out weights and x so every DMA is contiguous per
        # partition (the strided w6 slices measured ~26GB/s).  Small
        # critical slices (dc 0-1) first, on separate engine queues.
        wqk = wpool.tile([128, DC, 256], bf16, tag="wqk", name="wqk")
        wv = wpool.tile([128, DC, 128], bf16, tag="wv", name="wv")
        xts = []
        xt0 = xpool.tile([128, DC, 512], bf16, tag="xt", name="xt0")
        # Cross-queue DMA transfers execute roughly in ISSUE order, so the
        # critical first slices must be issued before any bulk: sync issues
        # earliest (x dc0-1), scalar next (Q/K weights), gpsimd the rest.
        nc.sync.dma_start(out=xt0[:, 0:2, :], in_=xh[:, 0, 0:2, :])
        nc.scalar.dma_start(out=wqk[:, 0:2, :], in_=w6qk[:, 0:2, :])
        nc.sync.dma_start(out=xt0[:, 2:4, :], in_=xh[:, 0, 2:4, :])
        nc.scalar.dma_start(out=wqk[:, 2:DC, :], in_=w6qk[:, 2:DC, :])
        nc.sync.dma_start(out=xt0[:, 4:DC, :], in_=xh[:, 0, 4:DC, :])
        nc.sync.dma_start(out=wv, in_=w6v)
        xts.append(xt0)

        # ---- constants (gpsimd; overlaps the DMAs) ----------------------
        # warm tile first: the PE warmup matmuls wait only on this memset
        warm = constp.tile([128, 512], bf16)
        nc.gpsimd.memset(warm, 0.0)
        id64 = constp.tile([128, 64], bf16)
        nc.gpsimd.memset(id64, 0.0)
        nc.gpsimd.affine_select(
            out=id64, in_=id64, compare_op=ne, fill=1.0,
            base=0, channel_multiplier=1, pattern=[[-1, 64]],
        )
        nc.gpsimd.affine_select(
            out=id64, in_=id64, compare_op=ne, fill=1.0,
            base=-64, channel_multiplier=1, pattern=[[-1, 64]],
        )
        biasap = constp.tile([128, 1], f32)
        nc.gpsimd.memset(biasap, ACT_BIAS)

        # ---- PE warmup during the input-DMA wait -----------------------
        # The PE drops to 1.2GHz after ~any idle and takes ~3us of
        # continuous work to reach 2.4GHz; dummy matmuls on a zeroed
        # scratch tile ramp it while the first x/w slices stream in.
        wps = pp.tile([64, 256], f32, tag="p", name="warm_ps")
        for _ in range(24):
            nc.tensor.matmul(wps, lhsT=warm[:, 0:64], rhs=warm[:, 0:256],
                             start=True, stop=True)

        # ---- persistent activations ------------------------------------
        QT = bigp.tile([128, T], bf16)
        KT = bigp.tile([128, T], bf16)
        Vext0 = bigp.tile([128, JBN, HS + 1], bf16)
        Vext1 = bigp.tile([128, JBN, HS + 1], bf16)
        onesb = constp.tile([128, JBN], bf16)
        nc.gpsimd.memset(onesb, 1.0)
        nc.vector.tensor_copy(Vext0[:, :, HS], onesb)
        nc.vector.tensor_copy(Vext1[:, :, HS], onesb)

        def prefetch_chunk(tcj):
            if tcj < TC and len(xts) <= tcj:
                xtn = xpool.tile([128, DC, 512], bf16, tag="xt", name=f"xt{tcj}")
                nc.sync.dma_start(out=xtn[:, 0:4, :], in_=xh[:, tcj, 0:4, :])
                nc.sync.dma_start(out=xtn[:, 4:8, :], in_=xh[:, tcj, 4:8, :])
                xts.append(xtn)

        def emit_proj_chunk(tcj):
            ts = slice(tcj * 512, (tcj + 1) * 512)
            # prefetch two chunks ahead: chunk k's ~5us of projection matmuls
            # barely covers one 1MB x-chunk DMA, so one-ahead slips ~1us/chunk.
            # (Chunk 0 prefetches only one ahead: its 1MB must not contend
            # with the critical startup slices.)
            prefetch_chunk(tcj + 1)
            if tcj >= 1:
                prefetch_chunk(tcj + 2)
            xt = xts[tcj]
            # Q and K interleaved over dc so each x dc-slice is consumed as
            # its DMA lands (matters for the DMA-gated first chunks).
            ps0 = pp.tile([128, 512], f32, tag="p", name=f"ps0_{tcj}")
            ps1 = pp.tile([128, 512], f32, tag="p", name=f"ps1_{tcj}")
            for dc in range(DC):
                for fc, ps in ((0, ps0), (1, ps1)):
                    nc.tensor.matmul(
                        ps,
                        lhsT=wqk[:, dc, fc * 128:(fc + 1) * 128],
                        rhs=xt[:, dc, :],
                        start=(dc == 0), stop=(dc == DC - 1),
                    )
            # PSUM->SBUF casts ride the Act engine: DVE is the busier
            # exp partner (1218ns/block vs Act's 1007ns).
            nc.scalar.copy(QT[:, ts], ps0)
            nc.scalar.copy(KT[:, ts], ps1)
            psv = pp.tile([128, 512], f32, tag="p", name=f"psv_{tcj}")
            for dc in range(DC):
                nc.tensor.matmul(
                    psv,
                    lhsT=wv[:, dc, :],
                    rhs=xt[:, dc, :],
                    start=(dc == 0), stop=(dc == DC - 1),
                )
            vts = vtp.tile([128, 512], bf16, tag="vts", name=f"vts_{tcj}")
            nc.scalar.copy(vts, psv)
            for h in range(HPC):
                vdst = Vext0 if h == 0 else Vext1
                ptv4 = pp.tile([128, 4, 64], bf16, tag="p", name=f"ptv{h}_{tcj}")
                for q in range(4):
                    nc.tensor.transpose(
                        ptv4[:, q, :],
                        in_=vts[h * 64:(h + 1) * 64, q * 128:(q + 1) * 128],
                        identity=id64[h * 64:(h + 1) * 64, :],
                        tile_position=(h * 64, 0),
                    )
                # one strided copy for all four j-blocks of this chunk
                nc.vector.tensor_copy(
                    vdst[:, tcj * 4:tcj * 4 + 4, 0:HS], ptv4)

        def emit_scores(ps, ib, jb, i0, w, name):
            """Both heads' S^T[j, i0:i0+w] for j-block jb into ps[:, h, i0:]."""
            gi0 = ib * 512 + i0
            for h in range(HPC):
                nc.tensor.matmul(
                    ps[:, h, i0:i0 + w],
                    lhsT=QT[h * 64:(h + 1) * 64, jb * 128:(jb + 1) * 128],
                    rhs=KT[h * 64:(h + 1) * 64, gi0:gi0 + w],
                    start=True, stop=True,
                    tile_position=(h * 64, 0),
                )

        def emit_attn_block(ib):
            isl = slice(ib * 512, (ib + 1) * 512)
            po = [
                op.tile([128, 512], f32, tag="o", name=f"po{h}_{ib}")
                for h in range(HPC)
            ]
            n_blocks = 4 * (ib + 1)
            state = {"started": False, "pending": [], "idx": 0}

            def flush_pending():
                for av in state["pending"]:
                    av()
                state["pending"] = []

            def push_pending(av):
                # Defer each AV two blocks behind its scores: the exp engine
                # gets ~2.7us of slack instead of ~100ns, so the PE never
                # waits on an exp in steady state.
                if len(state["pending"]) >= 3:
                    state["pending"].pop(0)()
                state["pending"].append(av)

            def do_block(jb, use_dve):
                ps = sp.tile([128, 2, 512], f32, tag="s", name=f"s_{ib}_{jb}")
                emit_scores(ps, ib, jb, 0, 512, f"s{ib}_{jb}")
                es = esp.tile([128, 2, 512], bf16, tag="es", name=f"es_{ib}_{jb}")
                if use_dve:
                    nc.vector.tensor_scalar(es.bitcast(i16), ps, A_TRICK,
                                            B_TRICK, mult, add)
                else:
                    nc.scalar.activation(es, ps, Exp, scale=SC, bias=biasap)
                first = not state["started"]
                state["started"] = True
                last = state["idx"] == n_blocks - 1
                state["idx"] += 1

                def av():
                    for h in range(HPC):
                        vdst = Vext0 if h == 0 else Vext1
                        nc.tensor.matmul(
                            po[h][0:65, :],
                            lhsT=vdst[:, jb, :],
                            rhs=es[:, h, :],
                            start=first, stop=last,
                        )
                push_pending(av)

            def do_diag(q, use_dve=False):
                jb = 4 * ib + q
                i0 = 128 * q
                w = 512 - i0
                ps = sp.tile([128, 2, 512], f32, tag="s", name=f"s_{ib}_{jb}")
                emit_scores(ps, ib, jb, i0, w, f"s{ib}_{jb}")
                es = esp.tile([128, 2, 512], bf16, tag="es", name=f"es_{ib}_{jb}")
                if use_dve:
                    nc.vector.tensor_scalar(es.bitcast(i16)[:, :, i0:],
                                            ps[:, :, i0:], A_TRICK, B_TRICK,
                                            mult, add)
                else:
                    nc.scalar.activation(es[:, :, i0:], ps[:, :, i0:], Exp,
                                         scale=SC, bias=biasap)
                # causal wedge: keep iff (i0 + ii) >= j + 128q  <=>  ii >= j
                nc.gpsimd.affine_select(
                    out=es[:, :, i0:], in_=es[:, :, i0:], compare_op=ge,
                    fill=0.0, base=0, channel_multiplier=-1,
                    pattern=[[0, 2], [1, w]],
                )
                first = not state["started"]
                state["started"] = True
                last = state["idx"] == n_blocks - 1
                state["idx"] += 1

                def av():
                    for h in range(HPC):
                        vdst = Vext0 if h == 0 else Vext1
                        nc.tensor.matmul(
                            po[h][0:65, i0:],
                            lhsT=vdst[:, jb, :],
                            rhs=es[:, h, i0:],
                            start=first, stop=last,
                        )
                push_pending(av)

            def offdiag(jp0=0, jp1=None):
                # Strict per-block Act/DVE alternation: both exp engines run
                # concurrently every pair, so the combined exp rate (~1.9
                # elem/ns) always exceeds the PE's ~1.1 elem/ns demand and the
                # PE never throttles to a single engine's pace.
                if jp1 is None:
                    jp1 = 2 * ib
                for jp in range(jp0, jp1):
                    do_block(2 * jp, use_dve=False)
                    do_block(2 * jp + 1, use_dve=True)

            def diag():
                for q in range(4):
                    do_diag(q, use_dve=(q % 2 == 1))

            def finish():
                flush_pending()
                attn_finish_tail(ib, po, state, flush_pending)

            return offdiag, diag, finish

        def attn_finish_tail(ib, po, state, flush_pending):
            for h in range(HPC):
                ot = finp.tile([65, 512], f32, tag="ot", name=f"ot{h}_{ib}")
                # h0 on Act, h1 on DVE; halves pipelined with the DMAs, each
                # head's DMAs on its own queue (sync / gpsimd) so issues
                # overlap.
                cp = nc.scalar.copy if h == 0 else nc.vector.tensor_copy
                dq = nc.sync if h == 0 else nc.gpsimd
                for q in range(2):
                    fsl = slice(q * 256, (q + 1) * 256)
                    osl = slice(ib * 512 + q * 256, ib * 512 + (q + 1) * 256)
                    cp(ot[:, fsl], po[h][0:65, fsl])
                    dq.dma_start(
                        out=out[h * 65:(h + 1) * 65, osl], in_=ot[:, fsl])

        # Staircase: attention block k needs projection chunks <= k (diag
        # needs exactly chunk k).  Emit proj chunk k+1 between the off-diag
        # and diag phases of attention block k: the PE chews projection
        # matmuls while the Act engine catches up on the off-diag exps,
        # instead of stalling behind the last pending AV.
        emit_proj_chunk(0)
        for k in range(TC):
            offdiag_k, diag_k, finish_k = emit_attn_block(k)
            if k + 1 < TC:
                # Proj chunk k+1 BEFORE block k's attention: its QT/KT/vts
                # copies land at the front of the Act queue, so block k+1's
                # scores never wait on them behind block k's exps.
                emit_proj_chunk(k + 1)
                offdiag_k()
                diag_k()
            else:
                # Last block: two off-diag pairs first (PE work for the
                # serial diag score->exp->mask->AV chain to hide behind),
                # then the diagonal, then the remaining pairs -- so neither
                # the diag chain nor the diag masks stick out as tail.
                offdiag_k(0, 2)
                diag_k()
                offdiag_k(2, 2 * k)
            finish_k()


_NO_HOIST_TYPES = frozenset({"InstNoOp"})


def _pair_ldweights(nc):
    """Reorder LDW0,MM0,LDW1,MM1 -> LDW0,LDW1,MM0,MM1 for row-group pairs."""
    for f in nc.m.functions:
        for blk in f.blocks:
            insts = blk.instructions
            changed = False
            i = 0
            while i + 3 < len(insts):
                a, b, c, d = insts[i:i + 4]
                if (
                    type(a).__name__ == "InstLdweights"
                    and type(b).__name__ == "InstMatmult"
                    and type(c).__name__ == "InstLdweights"
                    and type(d).__name__ == "InstMatmult"
                    and b.tile_position is not None
                    and c.tile_position is not None
                    and b.tile_position[0] == 0
                    and c.tile_position[0] == 64
                    and b.tile_size is not None
                    and b.tile_size[0] <= 64
                ):
                    insts[i + 1], insts[i + 2] = c, b
                    changed = True
                    i += 4
                else:
                    i += 1
            if changed:
                blk.instructions = insts


def _legalize_waits(nc):
    """Hoist multi-waits off engine instructions onto preceding NoOps."""
    import bass_rust

    for f in nc.m.functions:
        for blk in f.blocks:
            out = []
            changed = False
            for inst in blk.instructions:
                si = getattr(inst, "sync_info", None)
                if (
                    type(inst).__name__ not in _NO_HOIST_TYPES
                    and si is not None
                    and len(si.on_wait) >= 2
                ):
                    waits = list(si.on_wait)
                    for k, w in enumerate(waits[:-1]):
                        nop = bass_rust.InstNoOp(name=f"{inst.name}_hoistw{k}")
                        nop.engine = inst.engine
                        nop.sync_info = bass_rust.SyncInfo(
                            on_wait=[w], on_update=[]
                        )
                        out.append(nop)
                    si.on_wait = [waits[-1]]
                    changed = True
                out.append(inst)
            if changed:
                blk.instructions = out


def _build_program():
    import concourse.bass as bass
    import concourse.mybir as mybir
    import concourse.tile as tile

    nc = bass.Bass("TRN2", target_bir_lowering=False, debug=False, num_devices=NCORES)
    xh = nc.dram_tensor("xh", [128, TC, DC, 512], mybir.dt.bfloat16, kind="ExternalInput").ap()
    w6qk = nc.dram_tensor("w6qk", [128, DC, 256], mybir.dt.bfloat16, kind="ExternalInput").ap()
    w6v = nc.dram_tensor("w6v", [128, DC, 128], mybir.dt.bfloat16, kind="ExternalInput").ap()
    out = nc.dram_tensor("outR", [HPC * (HS + 1), T], mybir.dt.float32, kind="ExternalOutput").ap()

    with tile.TileContext(nc) as tc:
        _emit(tc, nc, xh, w6qk, w6v, out)
    _pair_ldweights(nc)
    _legalize_waits(nc)
    return nc


def _in_maps(x, Wk, Wq, Wv):
    import ml_dtypes

    bf = ml_dtypes.bfloat16
    # x host layout [128, TC, DC, 512]: xh[p, tc, dc, t] = x[tc*512+t, dc*128+p]
    xf = np.asarray(x, dtype=np.float32).astype(bf)
    xh = np.ascontiguousarray(
        xf.reshape(TC, 512, DC, 128).transpose(3, 0, 2, 1))
    maps = []
    for c in range(NCORES):
        h0, h1 = HPC * c, HPC * c + 1
        WQK = np.concatenate(
            [Wq[h0], Wq[h1], Wk[h0], Wk[h1]], axis=1).astype(bf)
        WV = np.concatenate([Wv[h0], Wv[h1]], axis=1).astype(bf)
        # [D, F] -> [128, DC, F]: w[p, dc, f] = W[dc*128+p, f]
        wqk = np.ascontiguousarray(WQK.reshape(DC, 128, 256).transpose(1, 0, 2))
        wv = np.ascontiguousarray(WV.reshape(DC, 128, 128).transpose(1, 0, 2))
        maps.append({"xh": xh, "w6qk": wqk, "w6v": wv})
    return maps


def get_program():
    global _cached_nc
    if _cached_nc is None:
        _cached_nc = _build_program()
    return _cached_nc


def kernel(x, Wk, Wq, Wv):
    import os

    from concourse.bass_utils import run_bass_kernel_spmd

    os.environ.setdefault("NEURON_FORCE_RECOMPILE", "1")
    os.environ.setdefault("NEURON_RT_RESET_CORES", "1")

    nc = get_program()
    res = run_bass_kernel_spmd(nc, _in_maps(x, Wk, Wq, Wv), core_ids=list(range(NCORES)))
    cols = []
    for c in range(NCORES):
        raw = res.results[c]["outR"]  # [2*65, T]: per head 64 rows O^T + denom
        for h in range(HPC):
            o = raw[h * 65:h * 65 + HS]
            den = raw[h * 65 + HS:h * 65 + HS + 1]
            cols.append((o / den).T)
    return np.ascontiguousarray(np.concatenate(cols, axis=1), dtype=np.float32)

